# revision 1
# baseline (speedup 1.0000x reference)
"""Trainium2 Bass kernel for nn_BaselineAttn (LoRA QKV + ALiBi causal attention).

Sharding: 8 cores SPMD, no collectives. Core c = (b, g): batch b = c // 4,
head group g = c % 4 handling heads [g, 4+g, 8+g, 12+g].

Host prep: LoRA folded into weights (W' = W + 2 A@B); x and weights
pre-transposed/sliced per core; partial outputs summed on host.

Device design (fp16 operands, fp32 PSUM):
  - feature-major x^T on chip -> q^T, k^T feature-major and v token-major
    from the same x^T; zero on-chip transposes.
  - attention in the S^T (key-major) orientation:
      S^T tile = k^T-tile.T @ q^T-chunk
      P^T = exp(S^T/8 + bias_k), bias_k = -slope_h*k per-PARTITION: ALiBi +
        softmax shift fused into one ScalarE activation.
      causal: diagonal-band tiles multiplied by a 0/1 mask; dead tiles
        skipped; per-tile active q-range sliced.
      O^T += (v|ones).T @ P^T  (ones column = softmax denominator in row 64)
      normalize: fast-reciprocal -> pair-merged PE broadcast -> DVE mul.
      out-partial = O^T_norm.T @ Wp'^T-slice.
  - ALiBi gives key k weight exp(-slope_h*k); keys with slope_h*k > ~45 are
    dropped (< 1e-16 of row mass). Head->slot assignment makes per-slot
    k-tile caps uniform across cores: SNKT = [2, 6, 16, 16].
"""

import math

import numpy as np

E = 1024
H = 16
DH = 64
T = 2048
BATCH = 2
LORA_S = 2.0
NKT = T // 128          # 16 key tiles of 128
SNKT = [2, 6, 16, 16]   # per-slot key-tile caps (max over cores per slot)
NQC = 4                 # q chunks of 512

_NC_CACHE = {}


def _slopes():
    start = 2 ** (-2 ** (-(math.log2(H) - 3)))
    return np.array([start * start**i for i in range(H)], dtype=np.float64)


def _smin(tt):
    """Lowest slot that still needs key-tile tt."""
    for s in range(4):
        if tt < SNKT[s]:
            return s
    return 4


def _build_nc():
    """Build the single SPMD Bass program (shared by all 8 cores)."""
    if "nc" in _NC_CACHE:
        return _NC_CACHE["nc"]

    from concourse.bacc import Bacc
    import concourse.tile as tile
    from concourse import mybir

    f16 = mybir.dt.float16
    f32 = mybir.dt.float32
    EXP = mybir.ActivationFunctionType.Exp

    nc = Bacc()

    xT_d = nc.dram_tensor("xT", [E, T], f16, kind="ExternalInput")
    wqkv_d = nc.dram_tensor("wqkvT", [E, 768], f16, kind="ExternalInput")
    wp_d = nc.dram_tensor("wpT", [256, E], f16, kind="ExternalInput")
    bias_d = nc.dram_tensor("expbias", [128, 64], f32, kind="ExternalInput")
    mask_d = nc.dram_tensor("masks", [128, 4 * 512], f16, kind="ExternalInput")
    ones_d = nc.dram_tensor("ones2", [2, 128], f16, kind="ExternalInput")
    out_d = nc.dram_tensor("outp", [T, E], f32, kind="ExternalOutput")
    rbounce_d = nc.dram_tensor("rbounce", [16, 512], f32, kind="Internal")

    with tile.TileContext(nc) as tc:
        with (
            tc.tile_pool(name="persist", bufs=1) as pp,
            tc.tile_pool(name="ptpool", bufs=10) as ptp,
            tc.tile_pool(name="onorm", bufs=4) as onp,
            tc.tile_pool(name="rpool", bufs=4) as rp,
            tc.tile_pool(name="outsb", bufs=6) as osp,
        ):
            # ---- input loads (interleaved so the first MMs start early) ----
            wqkv, xT = [], []
            for kt in range(8):
                w_t = pp.tile([128, 768], f16, name=f"wqkv{kt}")
                nc.sync.dma_start(out=w_t, in_=wqkv_d[kt * 128:(kt + 1) * 128, :])
                wqkv.append(w_t)
                x_t = pp.tile([128, T], f16, name=f"xT{kt}")
                nc.scalar.dma_start(out=x_t, in_=xT_d[kt * 128:(kt + 1) * 128, :])
                xT.append(x_t)
            wp = []
            for pt in range(2):
                wp_t = pp.tile([128, E], f16, name=f"wp{pt}")
                nc.sync.dma_start(out=wp_t, in_=wp_d[pt * 128:(pt + 1) * 128, :])
                wp.append(wp_t)
            bias_sb = pp.tile([128, 64], f32, name="bias")
            nc.sync.dma_start(out=bias_sb, in_=bias_d[:, :])
            mask_sb = pp.tile([128, 4 * 512], f16, name="mask")
            nc.sync.dma_start(out=mask_sb, in_=mask_d[:, :])
            ones_sb = pp.tile([2, 128], f16, name="ones2")
            nc.sync.dma_start(out=ones_sb, in_=ones_d[:, :])

            vext = []
            for tt in range(NKT):
                v_t = pp.tile([128, 4, 65], f16, name=f"vext{tt}")
                nc.gpsimd.memset(v_t, 1.0)  # ones cols preset; v overwrites rest
                vext.append(v_t)
            # q^T / k^T: per (p-tile, chunk) tiles [128, 512].
            # kT p-tile 0 (slots 0,1) only needs k < 768: chunks 2,3 never read.
            qT = [[pp.tile([128, 512], f16, name=f"qT{p}_{ncu}") for ncu in range(NQC)]
                  for p in range(2)]
            kT = [[pp.tile([128, 512], f16, name=f"kT{p}_{ncu}")
                   if (p == 1 or ncu < 2) else None for ncu in range(NQC)]
                  for p in range(2)]

            # ---- phase 1: QKV projections ----
            with tc.tile_pool(name="qkps", bufs=3, space="PSUM") as qkps, \
                 tc.tile_pool(name="vps", bufs=3, space="PSUM") as vps:
                # chunk-major emission: q,k,v for chunk ncu before chunk ncu+1,
                # so attention for q-chunk 0 can start 4x earlier.
                with nc.named_scope("qkv_proj"):
                    for ncu in range(NQC):
                        for wofs, dst in ((0, qT), (256, kT)):
                            for mt in range(2):
                                if dst[mt][ncu] is None:
                                    continue
                                # kT[0][1]: only k in [512, 768) used -> N=256
                                nw = 256 if (wofs == 256 and mt == 0 and ncu == 1) else 512
                                acc = qkps.tile([128, 512], f32, tag="qkacc",
                                                name=f"qkacc{wofs}_{mt}_{ncu}")
                                for kt in range(8):
                                    nc.tensor.matmul(
                                        acc[:, 0:nw],
                                        wqkv[kt][:, wofs + mt * 128:wofs + (mt + 1) * 128],
                                        xT[kt][:, ncu * 512:ncu * 512 + nw],
                                        start=(kt == 0), stop=(kt == 7),
                                    )
                                nc.scalar.copy(out=dst[mt][ncu][:, 0:nw],
                                               in_=acc[:, 0:nw])
                        for tt in range(4 * ncu, 4 * ncu + 4):
                            s0 = _smin(tt)
                            nw = (4 - s0) * 64
                            acc = vps.tile([128, 256], f32, tag="vacc", name=f"vacc{tt}")
                            for kt in range(8):
                                nc.tensor.matmul(
                                    acc[:, 0:nw],
                                    xT[kt][:, tt * 128:(tt + 1) * 128],
                                    wqkv[kt][:, 512 + s0 * 64:768],
                                    start=(kt == 0), stop=(kt == 7),
                                )
                            nc.scalar.copy(
                                out=vext[tt][:, s0:4, 0:64],
                                in_=acc[:, 0:nw].rearrange("p (s d) -> p s d", d=64))

            # ---- phase 2: attention + output projection, per q-chunk ----
            with tc.tile_pool(name="stps", bufs=3, space="PSUM") as stps, \
                 tc.tile_pool(name="otps", bufs=4, space="PSUM") as otps, \
                 tc.tile_pool(name="prps", bufs=1, space="PSUM") as prps:
                nmask = 0
                for qc in range(NQC):
                    on_tiles = [onp.tile([128, 512], f16, tag="on", name=f"on_{qc}_{p}")
                                for p in range(2)]
                    for pair in (1, 0):
                        recips = [None, None]
                        ot_save = [None, None]
                        for s in (2 * pair + 1, 2 * pair):
                            nkt = min(SNKT[s], 4 * qc + 4)
                            pt_i = pair
                            r0 = 64 * (s % 2)
                            ot = otps.tile([128, 512], f32, tag="ot", name=f"ot_{qc}_{s}")
                            with nc.named_scope(f"attn_q{qc}_s{s}"):
                                for kt in range(nkt):
                                    j0 = (kt - 4 * qc) * 128 if kt >= 4 * qc else 0
                                    st = stps.tile([128, 512], f32, tag="st",
                                                   name=f"st_{qc}_{s}_{kt}")
                                    nc.tensor.matmul(
                                        st[:, j0:512],
                                        kT[pt_i][kt // 4][r0:r0 + 64,
                                                          (kt % 4) * 128:(kt % 4 + 1) * 128],
                                        qT[pt_i][qc][r0:r0 + 64, j0:512],
                                        start=True, stop=True,
                                    )
                                    p_t = ptp.tile([128, 512], f16, tag="pt",
                                                   name=f"pt_{qc}_{s}_{kt}")
                                    nc.scalar.activation(
                                        out=p_t[:, j0:512], in_=st[:, j0:512],
                                        func=EXP,
                                        bias=bias_sb[:, s * 16 + kt:s * 16 + kt + 1],
                                        scale=0.125,
                                    )
                                    if kt >= 4 * qc:
                                        m = kt - 4 * qc
                                        nmask += 1
                                        meng = nc.vector if nmask % 2 else nc.gpsimd
                                        meng.tensor_mul(
                                            out=p_t[:, j0:512],
                                            in0=p_t[:, j0:512],
                                            in1=mask_sb[:, m * 512 + j0:(m + 1) * 512],
                                        )
                                    nc.tensor.matmul(
                                        ot[0:65, j0:512],
                                        vext[kt][:, s, :],
                                        p_t[:, j0:512],
                                        start=(kt == 0), stop=(kt == nkt - 1),
                                    )
                                # denominator -> reciprocal
                                sum_sb = rp.tile([1, 512], f32, tag="sumsb",
                                                 name=f"sum_{qc}_{s}")
                                nc.vector.tensor_copy(out=sum_sb, in_=ot[64:65, :])
                                recip32 = rp.tile([1, 512], f32, tag="recip32",
                                                  name=f"recip32_{qc}_{s}")
                                nc.vector.reciprocal_approx_fast(out=recip32, in_=sum_sb)
                                nc.sync.dma_start(
                                    out=rbounce_d[4 * qc + s:4 * qc + s + 1, :],
                                    in_=recip32)
                                ot_save[s % 2] = ot
                        # DMA partition-broadcast of 1/denominator + normalize
                        with nc.named_scope(f"norm_q{qc}_p{pair}"):
                            bcs = rp.tile([128, 512], f32, tag="bcs",
                                          name=f"bcs_{qc}_{pair}")
                            for s in (2 * pair, 2 * pair + 1):
                                r0 = 64 * (s % 2)
                                nc.sync.dma_start(
                                    out=bcs[r0:r0 + 64, :],
                                    in_=rbounce_d[4 * qc + s:4 * qc + s + 1, :]
                                    .to_broadcast([64, 512]))
                            for s in (2 * pair, 2 * pair + 1):
                                r0 = 64 * (s % 2)
                                nc.vector.tensor_mul(
                                    out=on_tiles[pair][r0:r0 + 64, :],
                                    in0=ot_save[s % 2][0:64, :],
                                    in1=bcs[r0:r0 + 64, :],
                                )
                    with nc.named_scope(f"proj_q{qc}"):
                        for tloc in range(4):
                            tt = qc * 4 + tloc
                            for ech in range(2):
                                pacc = prps.tile([128, 512], f32, tag="pacc",
                                                 name=f"pacc_{tt}_{ech}")
                                for pt_i in range(2):
                                    nc.tensor.matmul(
                                        pacc,
                                        on_tiles[pt_i][:, tloc * 128:(tloc + 1) * 128],
                                        wp[pt_i][:, ech * 512:(ech + 1) * 512],
                                        start=(pt_i == 0), stop=(pt_i == 1),
                                    )
                                osb = osp.tile([128, 512], f32, tag="osb",
                                               name=f"osb_{tt}_{ech}")
                                nc.vector.tensor_copy(out=osb, in_=pacc)
                                nc.sync.dma_start(
                                    out=out_d[tt * 128:(tt + 1) * 128,
                                              ech * 512:(ech + 1) * 512],
                                    in_=osb)

    nc.finalize()
    _NC_CACHE["nc"] = nc
    return nc


def _prep_core_inputs(x, Wq, Aq, Bq, Wk, Ak, Bk, Wv, Av, Bv, Wp):
    """Host-side prep: LoRA fold, transposes, per-core slices."""
    slopes = _slopes()
    wq_m = Wq.astype(np.float64) + LORA_S * (Aq.astype(np.float64) @ Bq.astype(np.float64))
    wk_m = Wk.astype(np.float64) + LORA_S * (Ak.astype(np.float64) @ Bk.astype(np.float64))
    wv_m = Wv.astype(np.float64) + LORA_S * (Av.astype(np.float64) @ Bv.astype(np.float64))

    # mask_m[p, j] = 1 if (m*128 + p) <= j else 0   (j in 0..511)
    p_i = np.arange(128)[:, None]
    j_i = np.arange(512)[None, :]
    masks = np.ascontiguousarray(np.concatenate(
        [((m * 128 + p_i) <= j_i).astype(np.float16) for m in range(4)], axis=1))
    # ones2: row0 selects slot-even (out rows 0:64), row1 slot-odd (64:128)
    ones2 = np.zeros((2, 128), dtype=np.float16)
    ones2[0, 0:64] = 1.0
    ones2[1, 64:128] = 1.0

    in_maps = []
    for c in range(8):
        b, g = divmod(c, 4)
        heads = [g, 4 + g, 8 + g, 12 + g]
        rows = np.concatenate([np.arange(h * DH, (h + 1) * DH) for h in heads])
        xT = np.ascontiguousarray(x[b].T.astype(np.float16))
        wqkvT = np.ascontiguousarray(np.concatenate(
            [wq_m[rows, :].T, wk_m[rows, :].T, wv_m[rows, :].T],
            axis=1).astype(np.float16))
        wpT = np.ascontiguousarray(Wp[:, rows].T.astype(np.float16))
        bias = np.zeros((128, 64), dtype=np.float32)
        for s, h in enumerate(heads):
            for kt in range(16):
                bias[:, s * 16 + kt] = -slopes[h] * (kt * 128 + np.arange(128))
        in_maps.append({
            "xT": xT, "wqkvT": wqkvT, "wpT": wpT,
            "expbias": bias, "masks": masks, "ones2": ones2,
        })
    return in_maps


def _run(in_maps, trace=False, **kw):
    from concourse.bass_utils import run_bass_kernel_spmd
    nc = _build_nc()
    return run_bass_kernel_spmd(nc, in_maps, core_ids=list(range(8)), trace=trace, **kw)


def kernel(x, Wq, Aq, Bq, Wk, Ak, Bk, Wv, Av, Bv, Wp):
    in_maps = _prep_core_inputs(x, Wq, Aq, Bq, Wk, Ak, Bk, Wv, Av, Bv, Wp)
    res = _run(in_maps)
    out = np.zeros((BATCH, T, E), dtype=np.float32)
    for c in range(8):
        out[c // 4] += res.results[c]["outp"]
    return out

